# revision 86
# baseline (speedup 1.0000x reference)
"""Trainium2 Bass kernel for nn_MultiHeadedAttention (B=2, S=2048, D=1024, H=16).

Sharding: batch (2) x head-groups (4) -> 8 cores. Core c handles batch c//4,
heads [4*(c%4), 4*(c%4)+4).

Software-pipelined emission around the Scalar engine's exp stream (128
tiles of [128 kpos, 1024 q], ~129us): PE interleaves scores (producer,
2x512-row matmuls per tile), PV (consumer, trailing LAG2 tiles through a
24-deep SBUF staging ring), and all projection / V / output-projection
work as priority-ordered filler. Note the Tile scheduler list-schedules
per engine by readiness, with emission order acting as priority.
Inputs stream as per-e-chunk DMAs ordered by first use; output partials
are emitted in bf16 (host sums partials in fp32).

Math/layout notes:
  qt holds head h's dk on partitions 64*(h%2)..+64 with the other 64 rows
  zeroed; kt packs two heads per 128 partitions (the zero rows of qt
  cancel the other head in the K=128 scores contraction). V is kept in
  [seq, feat] layout with a ones column per head so PV's 65th output row
  accumulates the softmax denominators for free. exp runs on ScalarE with
  the 1/sqrt(dk) scale folded in (no max-subtraction: |scores| <~ 8 is
  safe in fp32). Normalization per (head, J-block): copy the denominator
  row out of PSUM, reciprocal_approx_fast on DVE, partition_broadcast on
  GPSIMD, one DVE multiply. The tiny t-bias MLP ([B,1,1,1] -> [B,64]) is
  folded into the K projection bias on the host.
"""

import numpy as np

B, S, D, H, DK = 2, 2048, 1024, 16, 64
HPC = 4            # heads per core
DPC = HPC * DK     # 256 features per core
NCORES = 8

TRACE = False          # test harness sets True to capture an NTFF profile
LAST_EXEC_NS = None    # filled when TRACE
LAST_RESULTS = None

_BUILT = None


def _install_ntff_shim():
    """antenv.axon_hooks is absent in this image; recreate it so trace=True
    can ship NTFF profiles back through the axon tunnel."""
    import sys, types
    try:
        from antenv import axon_hooks  # noqa: F401
        return
    except ImportError:
        pass
    import antenv
    mod = types.ModuleType("antenv.axon_hooks")
    _hook = [None]
    mod.set_axon_ntff_profile_hook = lambda h: _hook.__setitem__(0, h)
    mod.get_axon_ntff_profile_hook = lambda: _hook[0]
    sys.modules["antenv.axon_hooks"] = mod
    antenv.axon_hooks = mod
    try:
        from trn_agent_boot.trn_boot import _ntff_profile_via_ctypes
        mod.set_axon_ntff_profile_hook(
            _ntff_profile_via_ctypes("/opt/axon/libaxon_pjrt.so"))
    except Exception:
        pass


def _build():
    """Build the per-core Bass graph (identical on all 8 cores)."""
    import concourse.tile as tile
    from concourse import mybir, bacc

    f32 = mybir.dt.float32
    bf16 = mybir.dt.bfloat16

    nc = bacc.Bacc()

    xq_t = nc.dram_tensor("xq_t", [D, S], bf16, kind="ExternalInput")
    xk_t = nc.dram_tensor("xk_t", [D, S], bf16, kind="ExternalInput")
    xv_t = nc.dram_tensor("xv_t", [D, S], bf16, kind="ExternalInput")
    wq_t = nc.dram_tensor("wq_t", [D, DPC], bf16, kind="ExternalInput")
    wk_t = nc.dram_tensor("wk_t", [D, DPC], bf16, kind="ExternalInput")
    wv_t = nc.dram_tensor("wv_t", [D, DPC], bf16, kind="ExternalInput")
    wo_t = nc.dram_tensor("wo_t", [DPC, D], bf16, kind="ExternalInput")
    bq2 = nc.dram_tensor("bq2", [HPC, 2 * DK], f32, kind="ExternalInput")
    bk2 = nc.dram_tensor("bk2", [2, 128], f32, kind="ExternalInput")
    bv1 = nc.dram_tensor("bv1", [1, DPC], f32, kind="ExternalInput")
    bo8 = nc.dram_tensor("bo8", [8, 128], f32, kind="ExternalInput")
    y_t = nc.dram_tensor("y_t", [D, S], bf16, kind="ExternalOutput")

    NE = D // 128   # 8 feature chunks
    NST = S // 128  # 16 seq k-tiles of 128
    NPST = 25       # exp staging depth (p tiles in flight)

    with tile.TileContext(nc) as tc:
        with tc.tile_pool(name="consts", bufs=1) as consts, \
             tc.tile_pool(name="persist", bufs=1) as persist, \
             tc.tile_pool(name="xq_pool", bufs=1) as xq_pool, \
             tc.tile_pool(name="xk_pool", bufs=2) as xk_pool, \
             tc.tile_pool(name="xv_pool", bufs=2) as xv_pool, \
             tc.tile_pool(name="oasb", bufs=2) as oa_pool, \
             tc.tile_pool(name="dnsb", bufs=1) as dn_pool, \
             tc.tile_pool(name="dbsb", bufs=2) as db_pool, \
             tc.tile_pool(name="ysb", bufs=4) as y_pool, \
             tc.tile_pool(name="sc_ps", bufs=2, space="PSUM") as sc_ps, \
             tc.tile_pool(name="o_ps", bufs=1, space="PSUM") as o_ps, \
             tc.tile_pool(name="f_ps", bufs=2, space="PSUM") as f_ps:

            # ---- persistent activations ----
            # qt: head h lives on partitions 64*(h%2)..+64 of slot h, rest
            # zeroed. kt: two heads packed per 128 partitions (slot m holds
            # heads 2m/2m+1) -- no zero rows needed on the kt side because
            # qt's zero rows cancel the other head in the K=128 contraction.
            qt_sb = persist.tile([128, HPC, S], bf16, tag="qt")
            kt_sb = persist.tile([128, 2, S], bf16, tag="kt")
            nc.gpsimd.memset(qt_sb[:, :, :], 0.0)
            v_sb = persist.tile([128, NST, HPC, DK + 1], bf16, tag="v")
            # pst split into two tile objects (even/odd slots): the
            # dependency tracker keeps reader lists per tile, so a slot
            # re-write waits on half as many outstanding PV reads
            pst_a = persist.tile([128, (NPST + 1) // 2, 1024], bf16,
                                 tag="psta")
            pst_b = persist.tile([128, NPST // 2, 1024], bf16, tag="pstb")

            def pst_ap(slot):
                t = pst_a if slot % 2 == 0 else pst_b
                return t[:, slot // 2, :]
            xa0_sb = persist.tile([128, 2, 1024], bf16, tag="xa0")
            xa1_sb = persist.tile([128, 2, 1024], bf16, tag="xa1")
            ones1 = consts.tile([128, 1], f32, tag="ones1")
            nc.vector.memset(ones1[:, :], 1.0)
            nc.vector.tensor_copy(
                v_sb[:, :, :, DK:DK + 1].rearrange("p a b c -> p (a b c)"),
                ones1[:, 0:1].broadcast_to([128, NST * HPC]))

            # ---- weights + inputs: DMA emission order is priority order ----
            wq_sb = consts.tile([128, NE, DPC], bf16, tag="wq")
            nc.sync.dma_start(wq_sb[:, :, :],
                              wq_t.rearrange("(e p) n -> p e n", p=128))
            x_tiles = {}

            def emit_x_dmas(name, pool, dram, b, halves=1):
                t = pool.tile([128, NE, 1024], bf16, tag=name, name=name)
                x_tiles[(name, b)] = t
                src = dram.rearrange("(e p) s -> p e s", p=128)
                cw = 1024 // halves
                for hf in range(halves):
                    for e in range(NE):
                        nc.sync.dma_start(
                            t[:, e, hf * cw:(hf + 1) * cw],
                            src[:, e, b * 1024 + hf * cw:b * 1024 + (hf + 1) * cw])

            emit_x_dmas("xq", xq_pool, xq_t, 0)
            wk_sb = consts.tile([128, NE, DPC], bf16, tag="wk")
            nc.sync.dma_start(wk_sb[:, :, :],
                              wk_t.rearrange("(e p) n -> p e n", p=128))
            bq_sb = consts.tile([128, HPC], f32, tag="bq")
            bk_sb = consts.tile([128, 2], f32, tag="bk")
            nc.sync.dma_start(bq_sb[:, :], bq2.rearrange("h p -> p h"))
            nc.sync.dma_start(bk_sb[:, :], bk2.rearrange("m p -> p m"))
            emit_x_dmas("xk", xk_pool, xk_t, 0)
            emit_x_dmas("xk", xk_pool, xk_t, 1)
            wv_sb = consts.tile([128, NE, DPC], bf16, tag="wv")
            nc.sync.dma_start(wv_sb[:, :, :],
                              wv_t.rearrange("(e p) n -> p e n", p=128))
            bv_bc = consts.tile([128, HPC, DK], f32, tag="bvb")
            nc.sync.dma_start(
                bv_bc.rearrange("p h d -> p (h d)"),
                bv1[0:1, :].broadcast_to([128, DPC]))
            emit_x_dmas("xv", xv_pool, xv_t, 0)
            emit_x_dmas("xv", xv_pool, xv_t, 1)
            wo_sb = consts.tile([128, 2, D], bf16, tag="wo")
            nc.sync.dma_start(wo_sb[:, :, :],
                              wo_t.rearrange("(f p) n -> p f n", p=128))
            bo_sb = consts.tile([128, 8], f32, tag="bo")
            nc.sync.dma_start(bo_sb[:, :], bo8.rearrange("o p -> p o"))
            # xq block 1 DMAs are deferred (xq_pool bufs=1, SBUF pressure):
            # emitted mid-driver once block-0 Q chains are in the queue.

            # ---- emission state ----
            est = {"pe": 7000.0, "sc": 0.0}
            xq_b1_emitted = [False]
            chain_done = set()   # ('q'|'k', block, m)
            v_done = set()
            pst_slot = {}
            o_tile = [None]

            def emit_chain(kind, b, m, pool="f"):
                if (kind, b, m) in chain_done:
                    return
                chain_done.add((kind, b, m))
                if kind == "q" and b == 1 and ("xq", 1) not in x_tiles:
                    xq_b1_emitted[0] = True
                    emit_x_dmas("xq", xq_pool, xq_t, 1)
                x_t = x_tiles[("xq" if kind == "q" else "xk", b)]
                w_sb = wq_sb if kind == "q" else wk_sb
                dst = qt_sb if kind == "q" else kt_sb
                bias = bq_sb if kind == "q" else bk_sb
                ms = slice(m * 128, m * 128 + 128)
                if pool == "sc":
                    # prologue: borrow an idle scores PSUM tile so two chains
                    # can drain arriving x chunks concurrently
                    t = sc_ps.tile([128, 1024], f32, tag="sc", name="scps")
                    halves = [t[:, 0:512], t[:, 512:1024]]
                else:
                    halves = [f_ps.tile([128, 512], f32, tag="f", name="fps"),
                              f_ps.tile([128, 512], f32, tag="f", name="fps")]

                def evac(half, ps):
                    sl = slice(b * 1024 + half * 512,
                               b * 1024 + half * 512 + 512)
                    if kind == "q":
                        nc.vector.tensor_scalar_add(
                            dst[0:64, 2 * m, sl], ps[0:64, :],
                            bias[0:64, 2 * m:2 * m + 1])
                        nc.vector.tensor_scalar_add(
                            dst[64:128, 2 * m + 1, sl], ps[64:128, :],
                            bias[64:128, 2 * m + 1:2 * m + 2])
                    else:
                        # packed kt: both heads in one full-width op
                        nc.vector.tensor_scalar_add(
                            dst[:, m, sl], ps[:, :], bias[:, m:m + 1])

                for e in range(NE):
                    for half, ps in enumerate(halves):
                        nc.tensor.matmul(ps[:, :], w_sb[:, e, ms],
                                         x_t[:, e, half * 512:half * 512 + 512],
                                         start=(e == 0), stop=(e == NE - 1))
                for half, ps in enumerate(halves):
                    evac(half, ps)
                est["pe"] += 16 * 213

            def emit_v(st):
                if st in v_done:
                    return
                v_done.add(st)
                b, loc = st // 8, st % 8
                x_t = x_tiles[("xv", b)]
                ps = f_ps.tile([128, 512], f32, tag="f", name="fps")
                for e in range(NE):
                    nc.tensor.matmul(ps[:, 0:256],
                                     x_t[:, e, loc * 128:(loc + 1) * 128],
                                     wv_sb[:, e, :],
                                     start=(e == 0), stop=(e == NE - 1))
                est["pe"] += 8 * 107
                nc.vector.tensor_tensor(
                    out=v_sb[:, st, :, 0:DK],
                    in0=ps[:, 0:256].rearrange("p (h d) -> p h d", h=HPC),
                    in1=bv_bc[:, :, :],
                    op=mybir.AluOpType.add)

            def emit_y(J, o, half, pool="f"):
                xa_sb = xa0_sb if J == 0 else xa1_sb
                jj = slice(half * 512, half * 512 + 512)
                if pool == "sc":
                    # tail: the scores ring is idle; borrow it for a deeper
                    # out-projection pipeline
                    ps = sc_ps.tile([128, 1024], f32, tag="sc",
                                    name="scps")[:, 0:512]
                else:
                    ps = f_ps.tile([128, 512], f32, tag="f", name="fps")
                for n, f in enumerate((1, 0)):
                    nc.tensor.matmul(ps[:, :], wo_sb[:, f, o * 128:(o + 1) * 128],
                                     xa_sb[:, f, jj],
                                     start=(n == 0), stop=(n == 1))
                est["pe"] += 2 * 213
                y_sb = y_pool.tile([128, 512], bf16, tag="y", name="ysb")
                if J == 1:
                    # tail: Scalar is idle once the exp backbone ends
                    nc.scalar.activation(y_sb[:, :], ps[:, :],
                                         mybir.ActivationFunctionType.Identity,
                                         bias=bo_sb[:, o:o + 1])
                else:
                    nc.vector.tensor_scalar_add(y_sb[:, :], ps[:, :],
                                                bo_sb[:, o:o + 1])
                oj = slice(J * 1024 + half * 512, J * 1024 + half * 512 + 512)
                nc.sync.dma_start(y_t[o * 128:(o + 1) * 128, oj], y_sb[:, :])

            def emit_scores_exp(u, U, i):
                J, h = U
                sc = sc_ps.tile([128, 1024], f32, tag="sc", name="scps")
                ks = slice(i * 128, (i + 1) * 128)
                for half in range(2):
                    jj = slice(J * 1024 + half * 512, J * 1024 + half * 512 + 512)
                    nc.tensor.matmul(sc[:, half * 512:half * 512 + 512],
                                     kt_sb[:, h // 2, ks], qt_sb[:, h, jj],
                                     start=True, stop=True)
                est["pe"] += 426
                slot = u % NPST
                pst_slot[(U, i)] = slot
                nc.scalar.activation(pst_ap(slot), sc[:, :],
                                     mybir.ActivationFunctionType.Exp,
                                     scale=0.125)
                est["sc"] = max(est["sc"], est["pe"] + 400) + 1077

            def emit_norm(U):
                # softmax denominators ride along as o_ps row 64; copy that
                # row out, broadcast it across 64 partitions on gpsimd, and
                # normalize with a single DVE divide (PSUM in0, SBUF in1).
                J, h = U
                xa_sb = xa0_sb if J == 0 else xa1_sb
                Js = slice(0, 1024)
                last = U == (1, 1)
                pb = 64 * (h % 2)
                # denominator row straight from PSUM so the reciprocal and
                # broadcast start as early as possible
                dn = dn_pool.tile([1, 1024], f32, tag="dn", name="dn")
                nc.vector.tensor_copy(dn[0:1, :], o_tile[0][DK:DK + 1, :])
                nc.vector.reciprocal_approx_fast(dn[0:1, :], dn[0:1, :])
                db = db_pool.tile([64, 1024], f32, tag="db", name="db")
                nc.gpsimd.partition_broadcast(db[:, :], dn[0:1, :])
                if last:
                    # no successor needs this o_ps slot: multiply out of PSUM
                    src_ap = o_tile[0][0:DK, :]
                else:
                    oa = oa_pool.tile([DK + 1, 1024], bf16, tag="oa", name="oa")
                    nc.vector.tensor_copy(oa[:, :], o_tile[0][:, :])
                    src_ap = oa[0:DK, :]
                nc.vector.tensor_tensor(
                    out=xa_sb[pb:pb + DK, h // 2, Js], in0=src_ap,
                    in1=db[:, :], op=mybir.AluOpType.mult)

            def emit_pv(U, i):
                J, h = U
                emit_v(i)
                if i == 0:
                    o_tile[0] = o_ps.tile([DK + 1, 1024], f32, tag="o",
                                          name="ops")
                slot = pst_slot[(U, i)]
                for half in range(2):
                    hs = slice(half * 512, half * 512 + 512)
                    nc.tensor.matmul(o_tile[0][:, hs], v_sb[:, i, h, :],
                                     pst_ap(slot)[:, hs],
                                     start=(i == 0), stop=(i == NST - 1))
                est["pe"] += 426
                if i == NST - 1:
                    emit_norm(U)

            # ---- filler queue: (ready_ns, fn) in strict FIFO order ----
            # ready = conservative DMA-landing estimate (cumulative bytes at
            # ~0.35 MiB/us behind a ~9us fixed runtime startup).
            from collections import deque
            filler = deque()
            filler.append((29500, lambda: emit_chain("k", 1, 0)))
            filler.append((29500, lambda: emit_chain("k", 1, 1)))
            for st in range(NST):
                ready = {0: 34000, 1: 37000, 2: 40000, 3: 43000}[st // 4]
                filler.append((ready, lambda st=st: emit_v(st)))
            filler.append((50000, lambda: emit_chain("q", 1, 0)))
            filler.append((50000, lambda: emit_chain("q", 1, 1)))

            def pop_filler_if_slack(aggressive=False):
                while filler:
                    ready, fn = filler[0]
                    if not aggressive and est["pe"] + 500 > est["sc"]:
                        break
                    if ready > est["pe"]:
                        break
                    filler.popleft()
                    fn()

            # ---- prologue: all four block-0 chains; pairs share the idle
            # scores-PSUM banks so both consume arriving x chunks in parallel
            emit_chain("q", 0, 0, pool="sc")
            emit_chain("q", 0, 1, pool="sc")
            emit_chain("k", 0, 0)
            emit_chain("k", 0, 1, pool="sc")

            # ---- backbone ----
            units = [(0, 0), (0, 1), (0, 2), (0, 3),
                     (1, 2), (1, 3), (1, 0), (1, 1)]
            exp_seq = [(U, i) for U in units for i in range(NST)]
            pv_seq = exp_seq
            expidx = {t: u for u, t in enumerate(exp_seq)}
            v_ready = {st: {0: 37500, 1: 39500}.get(st // 4, 44000)
                       for st in range(NST)}
            pc = [0]
            LAG2 = 14

            def pump_pv(u, force=False):
                npv = 0
                while pc[0] < len(pv_seq) and npv < 2:
                    Uv, iv = pv_seq[pc[0]]
                    need = u - LAG2
                    if not force and expidx[(Uv, iv)] > need:
                        break
                    if (not force and iv not in v_done
                            and v_ready[iv] > est["pe"]):
                        break
                    emit_pv(Uv, iv)
                    pc[0] += 1
                    npv += 1
                    if iv == NST - 1 and Uv == (0, 3):
                        for o in range(2):
                            for hf in range(2):
                                filler.append(
                                    (0, lambda o=o, hf=hf: emit_y(0, o, hf)))

            for u, (U, i) in enumerate(exp_seq):
                J, h = U
                # gates: chains this scores tile depends on
                emit_chain("q", J, h // 2)
                emit_chain("k", i // 8, h // 2)
                pump_pv(u)
                pop_filler_if_slack(aggressive=(u >= 112))
                emit_scores_exp(u, U, i)
                if u == 6 and not xq_b1_emitted[0]:
                    xq_b1_emitted[0] = True
                    emit_x_dmas("xq", xq_pool, xq_t, 1)

            # ---- epilogue: drain PV + filler, then final out-projection ----
            u = len(exp_seq)
            while pc[0] < len(pv_seq):
                pump_pv(u, force=True)
                pop_filler_if_slack(aggressive=True)
            while filler:
                _, fn = filler.popleft()
                fn()
            for o in range(2, 8):
                for hf in range(2):
                    emit_y(0, o, hf)
            for o in range(8):
                for hf in range(2):
                    emit_y(1, o, hf, pool=("sc" if (o + hf) % 2 else "f"))

    nc.finalize()
    return nc


def _get_built():
    global _BUILT
    if _BUILT is None:
        _BUILT = _build()
    return _BUILT


def kernel(**inputs):
    global LAST_EXEC_NS, LAST_RESULTS
    import ml_dtypes
    from concourse import bass_utils

    bf16 = ml_dtypes.bfloat16
    inp = {k: np.ascontiguousarray(np.asarray(v), dtype=np.float32)
           for k, v in inputs.items()}

    # host: t-bias MLP, folded into the K-projection bias
    t = inp["t"].reshape(B)
    h1 = np.maximum(inp["tW1"][:, 0][None, :] * t[:, None] + inp["tb1"][None, :], 0.0)
    t_bias = h1 @ inp["tW2"].T + inp["tb2"][None, :]          # [B, DK]

    in_maps = []
    for c in range(NCORES):
        b, g = c // 4, c % 4
        sl = slice(g * DPC, (g + 1) * DPC)
        bo_full = inp["bo"] if g == 0 else np.zeros(D, np.float32)
        in_maps.append({
            "xq_t": np.ascontiguousarray(inp["query"][b].T.astype(bf16)),
            "xk_t": np.ascontiguousarray(inp["key"][b].T.astype(bf16)),
            "xv_t": np.ascontiguousarray(inp["value"][b].T.astype(bf16)),
            "wq_t": np.ascontiguousarray(inp["Wq"][sl, :].T.astype(bf16)),
            "wk_t": np.ascontiguousarray(inp["Wk"][sl, :].T.astype(bf16)),
            "wv_t": np.ascontiguousarray(inp["Wv"][sl, :].T.astype(bf16)),
            "wo_t": np.ascontiguousarray(inp["Wo"][:, sl].T.astype(bf16)),
            "bq2": np.tile(inp["bq"][sl].reshape(HPC, DK), (1, 2)),
            "bk2": (inp["bk"][sl] + np.tile(t_bias[b], HPC)).reshape(2, 128),
            "bv1": inp["bv"][sl].reshape(1, DPC).copy(),
            "bo8": bo_full.reshape(8, 128).copy(),
        })

    nc = _get_built()
    if TRACE:
        _install_ntff_shim()
    try:
        res = bass_utils.run_bass_kernel_spmd(
            nc, in_maps, core_ids=list(range(NCORES)), trace=TRACE)
    except Exception:
        # transient device-unrecoverable states have been observed on a
        # first run; one retry on a fresh execute context clears them
        import time
        time.sleep(2.0)
        res = bass_utils.run_bass_kernel_spmd(
            nc, in_maps, core_ids=list(range(NCORES)), trace=False)
    LAST_EXEC_NS = res.exec_time_ns
    LAST_RESULTS = res

    out = np.zeros((B, S, D), np.float32)
    for c in range(NCORES):
        out[c // 4] += res.results[c]["y_t"].astype(np.float32).T
    return out



# revision 87
# speedup vs baseline: 1.1885x; 1.1885x over previous
"""Trainium2 Bass kernel for nn_MultiHeadedAttention (B=2, S=2048, D=1024, H=16).

Sharding: batch (2) x head-groups (4) -> 8 cores. Core c handles batch c//4,
heads [4*(c%4), 4*(c%4)+4).

Software-pipelined emission around the Scalar engine's exp stream (128
tiles of [128 kpos, 1024 q], ~129us): PE interleaves scores (producer,
2x512-row matmuls per tile), PV (consumer, trailing LAG2 tiles through a
24-deep SBUF staging ring), and all projection / V / output-projection
work as priority-ordered filler. Note the Tile scheduler list-schedules
per engine by readiness, with emission order acting as priority.
Inputs stream as per-e-chunk DMAs ordered by first use; output partials
are emitted in bf16 (host sums partials in fp32).

Math/layout notes:
  qt holds head h's dk on partitions 64*(h%2)..+64 with the other 64 rows
  zeroed; kt packs two heads per 128 partitions (the zero rows of qt
  cancel the other head in the K=128 scores contraction). V is kept in
  [seq, feat] layout with a ones column per head so PV's 65th output row
  accumulates the softmax denominators for free. exp runs on ScalarE with
  the 1/sqrt(dk) scale folded in (no max-subtraction: |scores| <~ 8 is
  safe in fp32). Normalization per (head, J-block): copy the denominator
  row out of PSUM, reciprocal_approx_fast on DVE, partition_broadcast on
  GPSIMD, one DVE multiply. The tiny t-bias MLP ([B,1,1,1] -> [B,64]) is
  folded into the K projection bias on the host.
"""

import numpy as np

B, S, D, H, DK = 2, 2048, 1024, 16, 64
HPC = 4            # heads per core
DPC = HPC * DK     # 256 features per core
NCORES = 8

TRACE = False          # test harness sets True to capture an NTFF profile
LAST_EXEC_NS = None    # filled when TRACE
LAST_RESULTS = None

_BUILT = None


def _install_ntff_shim():
    """antenv.axon_hooks is absent in this image; recreate it so trace=True
    can ship NTFF profiles back through the axon tunnel."""
    import sys, types
    try:
        from antenv import axon_hooks  # noqa: F401
        return
    except ImportError:
        pass
    import antenv
    mod = types.ModuleType("antenv.axon_hooks")
    _hook = [None]
    mod.set_axon_ntff_profile_hook = lambda h: _hook.__setitem__(0, h)
    mod.get_axon_ntff_profile_hook = lambda: _hook[0]
    sys.modules["antenv.axon_hooks"] = mod
    antenv.axon_hooks = mod
    try:
        from trn_agent_boot.trn_boot import _ntff_profile_via_ctypes
        mod.set_axon_ntff_profile_hook(
            _ntff_profile_via_ctypes("/opt/axon/libaxon_pjrt.so"))
    except Exception:
        pass


def _build():
    """Build the per-core Bass graph (identical on all 8 cores)."""
    import concourse.tile as tile
    from concourse import mybir, bacc

    f32 = mybir.dt.float32
    bf16 = mybir.dt.bfloat16

    nc = bacc.Bacc()

    xq_t = nc.dram_tensor("xq_t", [D, S], bf16, kind="ExternalInput")
    xk_t = nc.dram_tensor("xk_t", [D, S], bf16, kind="ExternalInput")
    xv_t = nc.dram_tensor("xv_t", [D, S], bf16, kind="ExternalInput")
    wq_t = nc.dram_tensor("wq_t", [D, DPC], bf16, kind="ExternalInput")
    wk_t = nc.dram_tensor("wk_t", [D, DPC], bf16, kind="ExternalInput")
    wv_t = nc.dram_tensor("wv_t", [D, DPC], bf16, kind="ExternalInput")
    wo_t = nc.dram_tensor("wo_t", [DPC, D], bf16, kind="ExternalInput")
    bq2 = nc.dram_tensor("bq2", [HPC, 2 * DK], f32, kind="ExternalInput")
    bk2 = nc.dram_tensor("bk2", [2, 128], f32, kind="ExternalInput")
    bv1 = nc.dram_tensor("bv1", [1, DPC], f32, kind="ExternalInput")
    bo8 = nc.dram_tensor("bo8", [8, 128], f32, kind="ExternalInput")
    y_t = nc.dram_tensor("y_t", [D, S], bf16, kind="ExternalOutput")

    NE = D // 128   # 8 feature chunks
    NST = S // 128  # 16 seq k-tiles of 128
    NPST = 24       # exp staging depth (p tiles in flight)

    with tile.TileContext(nc) as tc:
        with tc.tile_pool(name="consts", bufs=1) as consts, \
             tc.tile_pool(name="persist", bufs=1) as persist, \
             tc.tile_pool(name="xq_pool", bufs=1) as xq_pool, \
             tc.tile_pool(name="xk_pool", bufs=2) as xk_pool, \
             tc.tile_pool(name="xv_pool", bufs=2) as xv_pool, \
             tc.tile_pool(name="oasb", bufs=2) as oa_pool, \
             tc.tile_pool(name="dnsb", bufs=1) as dn_pool, \
             tc.tile_pool(name="dbsb", bufs=2) as db_pool, \
             tc.tile_pool(name="ysb", bufs=4) as y_pool, \
             tc.tile_pool(name="sc_ps", bufs=2, space="PSUM") as sc_ps, \
             tc.tile_pool(name="o_ps", bufs=1, space="PSUM") as o_ps, \
             tc.tile_pool(name="f_ps", bufs=2, space="PSUM") as f_ps:

            # ---- persistent activations ----
            # qt: head h lives on partitions 64*(h%2)..+64 of slot h, rest
            # zeroed. kt: two heads packed per 128 partitions (slot m holds
            # heads 2m/2m+1) -- no zero rows needed on the kt side because
            # qt's zero rows cancel the other head in the K=128 contraction.
            qt_sb = persist.tile([128, HPC, S], bf16, tag="qt")
            kt_sb = persist.tile([128, 2, S], bf16, tag="kt")
            nc.gpsimd.memset(qt_sb[:, :, :], 0.0)
            v_sb = persist.tile([128, NST, HPC, DK + 1], bf16, tag="v")
            # pst split into two tile objects (even/odd slots): the
            # dependency tracker keeps reader lists per tile, so a slot
            # re-write waits on half as many outstanding PV reads
            pst_a = persist.tile([128, NPST // 2, 1024], bf16, tag="psta")
            pst_b = persist.tile([128, NPST // 2, 1024], bf16, tag="pstb")

            def pst_ap(slot):
                t = pst_a if slot % 2 == 0 else pst_b
                return t[:, slot // 2, :]
            xa0_sb = persist.tile([128, 2, 1024], bf16, tag="xa0")
            xa1_sb = persist.tile([128, 2, 1024], bf16, tag="xa1")
            ones1 = consts.tile([128, 1], f32, tag="ones1")
            nc.vector.memset(ones1[:, :], 1.0)
            nc.vector.tensor_copy(
                v_sb[:, :, :, DK:DK + 1].rearrange("p a b c -> p (a b c)"),
                ones1[:, 0:1].broadcast_to([128, NST * HPC]))

            # ---- weights + inputs: DMA emission order is priority order ----
            wq_sb = consts.tile([128, NE, DPC], bf16, tag="wq")
            nc.sync.dma_start(wq_sb[:, :, :],
                              wq_t.rearrange("(e p) n -> p e n", p=128))
            x_tiles = {}

            def emit_x_dmas(name, pool, dram, b, halves=1):
                t = pool.tile([128, NE, 1024], bf16, tag=name, name=name)
                x_tiles[(name, b)] = t
                src = dram.rearrange("(e p) s -> p e s", p=128)
                cw = 1024 // halves
                for hf in range(halves):
                    for e in range(NE):
                        nc.sync.dma_start(
                            t[:, e, hf * cw:(hf + 1) * cw],
                            src[:, e, b * 1024 + hf * cw:b * 1024 + (hf + 1) * cw])

            emit_x_dmas("xq", xq_pool, xq_t, 0)
            wk_sb = consts.tile([128, NE, DPC], bf16, tag="wk")
            nc.sync.dma_start(wk_sb[:, :, :],
                              wk_t.rearrange("(e p) n -> p e n", p=128))
            bq_sb = consts.tile([128, HPC], f32, tag="bq")
            bk_sb = consts.tile([128, 2], f32, tag="bk")
            nc.sync.dma_start(bq_sb[:, :], bq2.rearrange("h p -> p h"))
            nc.sync.dma_start(bk_sb[:, :], bk2.rearrange("m p -> p m"))
            emit_x_dmas("xk", xk_pool, xk_t, 0)
            emit_x_dmas("xk", xk_pool, xk_t, 1)
            wv_sb = consts.tile([128, NE, DPC], bf16, tag="wv")
            nc.sync.dma_start(wv_sb[:, :, :],
                              wv_t.rearrange("(e p) n -> p e n", p=128))
            bv_bc = consts.tile([128, HPC, DK], f32, tag="bvb")
            nc.sync.dma_start(
                bv_bc.rearrange("p h d -> p (h d)"),
                bv1[0:1, :].broadcast_to([128, DPC]))
            emit_x_dmas("xv", xv_pool, xv_t, 0)
            emit_x_dmas("xv", xv_pool, xv_t, 1)
            wo_sb = consts.tile([128, 2, D], bf16, tag="wo")
            nc.sync.dma_start(wo_sb[:, :, :],
                              wo_t.rearrange("(f p) n -> p f n", p=128))
            bo_sb = consts.tile([128, 8], f32, tag="bo")
            nc.sync.dma_start(bo_sb[:, :], bo8.rearrange("o p -> p o"))
            # xq block 1 DMAs are deferred (xq_pool bufs=1, SBUF pressure):
            # emitted mid-driver once block-0 Q chains are in the queue.

            # ---- emission state ----
            est = {"pe": 7000.0, "sc": 0.0}
            xq_b1_emitted = [False]
            chain_done = set()   # ('q'|'k', block, m)
            v_done = set()
            pst_slot = {}
            o_tile = [None]

            def emit_chain(kind, b, m, pool="f"):
                if (kind, b, m) in chain_done:
                    return
                chain_done.add((kind, b, m))
                if kind == "q" and b == 1 and ("xq", 1) not in x_tiles:
                    xq_b1_emitted[0] = True
                    emit_x_dmas("xq", xq_pool, xq_t, 1)
                x_t = x_tiles[("xq" if kind == "q" else "xk", b)]
                w_sb = wq_sb if kind == "q" else wk_sb
                dst = qt_sb if kind == "q" else kt_sb
                bias = bq_sb if kind == "q" else bk_sb
                ms = slice(m * 128, m * 128 + 128)
                if pool == "sc":
                    # prologue: borrow an idle scores PSUM tile so two chains
                    # can drain arriving x chunks concurrently
                    t = sc_ps.tile([128, 1024], f32, tag="sc", name="scps")
                    halves = [t[:, 0:512], t[:, 512:1024]]
                else:
                    halves = [f_ps.tile([128, 512], f32, tag="f", name="fps"),
                              f_ps.tile([128, 512], f32, tag="f", name="fps")]

                def evac(half, ps):
                    sl = slice(b * 1024 + half * 512,
                               b * 1024 + half * 512 + 512)
                    if kind == "q":
                        nc.vector.tensor_scalar_add(
                            dst[0:64, 2 * m, sl], ps[0:64, :],
                            bias[0:64, 2 * m:2 * m + 1])
                        nc.vector.tensor_scalar_add(
                            dst[64:128, 2 * m + 1, sl], ps[64:128, :],
                            bias[64:128, 2 * m + 1:2 * m + 2])
                    else:
                        # packed kt: both heads in one full-width op
                        nc.vector.tensor_scalar_add(
                            dst[:, m, sl], ps[:, :], bias[:, m:m + 1])

                for e in range(NE):
                    for half, ps in enumerate(halves):
                        nc.tensor.matmul(ps[:, :], w_sb[:, e, ms],
                                         x_t[:, e, half * 512:half * 512 + 512],
                                         start=(e == 0), stop=(e == NE - 1))
                for half, ps in enumerate(halves):
                    evac(half, ps)
                est["pe"] += 16 * 213

            def emit_v(st):
                if st in v_done:
                    return
                v_done.add(st)
                b, loc = st // 8, st % 8
                x_t = x_tiles[("xv", b)]
                ps = f_ps.tile([128, 512], f32, tag="f", name="fps")
                for e in range(NE):
                    nc.tensor.matmul(ps[:, 0:256],
                                     x_t[:, e, loc * 128:(loc + 1) * 128],
                                     wv_sb[:, e, :],
                                     start=(e == 0), stop=(e == NE - 1))
                est["pe"] += 8 * 107
                nc.vector.tensor_tensor(
                    out=v_sb[:, st, :, 0:DK],
                    in0=ps[:, 0:256].rearrange("p (h d) -> p h d", h=HPC),
                    in1=bv_bc[:, :, :],
                    op=mybir.AluOpType.add)

            def emit_y(J, o, half, pool="f"):
                xa_sb = xa0_sb if J == 0 else xa1_sb
                jj = slice(half * 512, half * 512 + 512)
                if pool == "sc":
                    # tail: the scores ring is idle; borrow it for a deeper
                    # out-projection pipeline
                    ps = sc_ps.tile([128, 1024], f32, tag="sc",
                                    name="scps")[:, 0:512]
                else:
                    ps = f_ps.tile([128, 512], f32, tag="f", name="fps")
                for n, f in enumerate((1, 0)):
                    nc.tensor.matmul(ps[:, :], wo_sb[:, f, o * 128:(o + 1) * 128],
                                     xa_sb[:, f, jj],
                                     start=(n == 0), stop=(n == 1))
                est["pe"] += 2 * 213
                y_sb = y_pool.tile([128, 512], bf16, tag="y", name="ysb")
                if J == 1:
                    # tail: Scalar is idle once the exp backbone ends
                    nc.scalar.activation(y_sb[:, :], ps[:, :],
                                         mybir.ActivationFunctionType.Identity,
                                         bias=bo_sb[:, o:o + 1])
                else:
                    nc.vector.tensor_scalar_add(y_sb[:, :], ps[:, :],
                                                bo_sb[:, o:o + 1])
                oj = slice(J * 1024 + half * 512, J * 1024 + half * 512 + 512)
                nc.sync.dma_start(y_t[o * 128:(o + 1) * 128, oj], y_sb[:, :])

            def emit_scores_exp(u, U, i):
                J, h = U
                sc = sc_ps.tile([128, 1024], f32, tag="sc", name="scps")
                ks = slice(i * 128, (i + 1) * 128)
                for half in range(2):
                    jj = slice(J * 1024 + half * 512, J * 1024 + half * 512 + 512)
                    nc.tensor.matmul(sc[:, half * 512:half * 512 + 512],
                                     kt_sb[:, h // 2, ks], qt_sb[:, h, jj],
                                     start=True, stop=True)
                est["pe"] += 426
                slot = u % NPST
                pst_slot[(U, i)] = slot
                nc.scalar.activation(pst_ap(slot), sc[:, :],
                                     mybir.ActivationFunctionType.Exp,
                                     scale=0.125)
                est["sc"] = max(est["sc"], est["pe"] + 400) + 1077

            def emit_norm(U):
                # softmax denominators ride along as o_ps row 64; copy that
                # row out, broadcast it across 64 partitions on gpsimd, and
                # normalize with a single DVE divide (PSUM in0, SBUF in1).
                J, h = U
                xa_sb = xa0_sb if J == 0 else xa1_sb
                Js = slice(0, 1024)
                last = U == (1, 1)
                pb = 64 * (h % 2)
                # denominator row straight from PSUM so the reciprocal and
                # broadcast start as early as possible
                dn = dn_pool.tile([1, 1024], f32, tag="dn", name="dn")
                nc.vector.tensor_copy(dn[0:1, :], o_tile[0][DK:DK + 1, :])
                nc.vector.reciprocal_approx_fast(dn[0:1, :], dn[0:1, :])
                db = db_pool.tile([64, 1024], f32, tag="db", name="db")
                nc.gpsimd.partition_broadcast(db[:, :], dn[0:1, :])
                if last:
                    # no successor needs this o_ps slot: multiply out of PSUM
                    src_ap = o_tile[0][0:DK, :]
                else:
                    oa = oa_pool.tile([DK + 1, 1024], bf16, tag="oa", name="oa")
                    nc.vector.tensor_copy(oa[:, :], o_tile[0][:, :])
                    src_ap = oa[0:DK, :]
                nc.vector.tensor_tensor(
                    out=xa_sb[pb:pb + DK, h // 2, Js], in0=src_ap,
                    in1=db[:, :], op=mybir.AluOpType.mult)

            def emit_pv(U, i):
                J, h = U
                emit_v(i)
                if i == 0:
                    o_tile[0] = o_ps.tile([DK + 1, 1024], f32, tag="o",
                                          name="ops")
                slot = pst_slot[(U, i)]
                for half in range(2):
                    hs = slice(half * 512, half * 512 + 512)
                    nc.tensor.matmul(o_tile[0][:, hs], v_sb[:, i, h, :],
                                     pst_ap(slot)[:, hs],
                                     start=(i == 0), stop=(i == NST - 1))
                est["pe"] += 426
                if i == NST - 1:
                    emit_norm(U)

            # ---- filler queue: (ready_ns, fn) in strict FIFO order ----
            # ready = conservative DMA-landing estimate (cumulative bytes at
            # ~0.35 MiB/us behind a ~9us fixed runtime startup).
            from collections import deque
            filler = deque()
            filler.append((29500, lambda: emit_chain("k", 1, 0)))
            filler.append((29500, lambda: emit_chain("k", 1, 1)))
            for st in range(NST):
                ready = {0: 34000, 1: 37000, 2: 40000, 3: 43000}[st // 4]
                filler.append((ready, lambda st=st: emit_v(st)))
            filler.append((50000, lambda: emit_chain("q", 1, 0)))
            filler.append((50000, lambda: emit_chain("q", 1, 1)))

            def pop_filler_if_slack(aggressive=False):
                while filler:
                    ready, fn = filler[0]
                    if not aggressive and est["pe"] + 500 > est["sc"]:
                        break
                    if ready > est["pe"]:
                        break
                    filler.popleft()
                    fn()

            # ---- prologue: all four block-0 chains; pairs share the idle
            # scores-PSUM banks so both consume arriving x chunks in parallel
            emit_chain("q", 0, 0, pool="sc")
            emit_chain("q", 0, 1, pool="sc")
            emit_chain("k", 0, 0)
            emit_chain("k", 0, 1, pool="sc")

            # ---- backbone ----
            units = [(0, 0), (0, 1), (0, 2), (0, 3),
                     (1, 2), (1, 3), (1, 0), (1, 1)]
            exp_seq = [(U, i) for U in units for i in range(NST)]
            pv_seq = exp_seq
            expidx = {t: u for u, t in enumerate(exp_seq)}
            v_ready = {st: {0: 37500, 1: 39500}.get(st // 4, 44000)
                       for st in range(NST)}
            pc = [0]
            LAG2 = 14

            def pump_pv(u, force=False):
                npv = 0
                while pc[0] < len(pv_seq) and npv < 2:
                    Uv, iv = pv_seq[pc[0]]
                    need = u - LAG2
                    if not force and expidx[(Uv, iv)] > need:
                        break
                    if (not force and iv not in v_done
                            and v_ready[iv] > est["pe"]):
                        break
                    emit_pv(Uv, iv)
                    pc[0] += 1
                    npv += 1
                    if iv == NST - 1 and Uv == (0, 3):
                        for o in range(2):
                            for hf in range(2):
                                filler.append(
                                    (0, lambda o=o, hf=hf: emit_y(0, o, hf)))

            for u, (U, i) in enumerate(exp_seq):
                J, h = U
                # gates: chains this scores tile depends on
                emit_chain("q", J, h // 2)
                emit_chain("k", i // 8, h // 2)
                pump_pv(u)
                pop_filler_if_slack(aggressive=(u >= 112))
                emit_scores_exp(u, U, i)
                if u == 6 and not xq_b1_emitted[0]:
                    xq_b1_emitted[0] = True
                    emit_x_dmas("xq", xq_pool, xq_t, 1)

            # ---- epilogue: drain PV + filler, then final out-projection ----
            u = len(exp_seq)
            while pc[0] < len(pv_seq):
                pump_pv(u, force=True)
                pop_filler_if_slack(aggressive=True)
            while filler:
                _, fn = filler.popleft()
                fn()
            for o in range(2, 8):
                for hf in range(2):
                    emit_y(0, o, hf)
            for o in range(8):
                for hf in range(2):
                    emit_y(1, o, hf, pool=("sc" if (o + hf) % 2 else "f"))

    nc.finalize()
    return nc


def _get_built():
    global _BUILT
    if _BUILT is None:
        _BUILT = _build()
    return _BUILT


def kernel(**inputs):
    global LAST_EXEC_NS, LAST_RESULTS
    import ml_dtypes
    from concourse import bass_utils

    bf16 = ml_dtypes.bfloat16
    inp = {k: np.ascontiguousarray(np.asarray(v), dtype=np.float32)
           for k, v in inputs.items()}

    # host: t-bias MLP, folded into the K-projection bias
    t = inp["t"].reshape(B)
    h1 = np.maximum(inp["tW1"][:, 0][None, :] * t[:, None] + inp["tb1"][None, :], 0.0)
    t_bias = h1 @ inp["tW2"].T + inp["tb2"][None, :]          # [B, DK]

    in_maps = []
    for c in range(NCORES):
        b, g = c // 4, c % 4
        sl = slice(g * DPC, (g + 1) * DPC)
        bo_full = inp["bo"] if g == 0 else np.zeros(D, np.float32)
        in_maps.append({
            "xq_t": np.ascontiguousarray(inp["query"][b].T.astype(bf16)),
            "xk_t": np.ascontiguousarray(inp["key"][b].T.astype(bf16)),
            "xv_t": np.ascontiguousarray(inp["value"][b].T.astype(bf16)),
            "wq_t": np.ascontiguousarray(inp["Wq"][sl, :].T.astype(bf16)),
            "wk_t": np.ascontiguousarray(inp["Wk"][sl, :].T.astype(bf16)),
            "wv_t": np.ascontiguousarray(inp["Wv"][sl, :].T.astype(bf16)),
            "wo_t": np.ascontiguousarray(inp["Wo"][:, sl].T.astype(bf16)),
            "bq2": np.tile(inp["bq"][sl].reshape(HPC, DK), (1, 2)),
            "bk2": (inp["bk"][sl] + np.tile(t_bias[b], HPC)).reshape(2, 128),
            "bv1": inp["bv"][sl].reshape(1, DPC).copy(),
            "bo8": bo_full.reshape(8, 128).copy(),
        })

    nc = _get_built()
    if TRACE:
        _install_ntff_shim()
    try:
        res = bass_utils.run_bass_kernel_spmd(
            nc, in_maps, core_ids=list(range(NCORES)), trace=TRACE)
    except Exception:
        # transient device-unrecoverable states have been observed on a
        # first run; one retry on a fresh execute context clears them
        import time
        time.sleep(2.0)
        res = bass_utils.run_bass_kernel_spmd(
            nc, in_maps, core_ids=list(range(NCORES)), trace=False)
    LAST_EXEC_NS = res.exec_time_ns
    LAST_RESULTS = res

    out = np.zeros((B, S, D), np.float32)
    for c in range(NCORES):
        out[c // 4] += res.results[c]["y_t"].astype(np.float32).T
    return out

